# revision 13
# baseline (speedup 1.0000x reference)
"""Trainium2 Bass kernel for nn_HDCNN (4-layer hyperbolic dilated CNN).

Data-parallel over 8 NeuronCores (4096 rows each), feature-transposed
bf16 layout: activations live as [feature, batch] tiles so the 64-tap
full convolution becomes banded matmuls with static weights.

Math (validated against the reference): the Poincare projection always
triggers (row norms >> atanh(maxnorm)), so tanh cancels and each layer
reduces to   out = m * relu(conv_u + delta * y)   with per-sample scalars
from s = ||conv_u||^2 and d = <conv_u, y>. The per-sample scale m is
scale-invariant layer to layer, so only the last layer's m is applied.

v4 (this file), ~4.7x faster on hardware than the v2 baseline:
- gpsimd (Pool) removed from all per-chunk work: measured ~2.7-6x slower
  per op than DVE on real hw and it serializes the z-phase.
- z = conv + delta*y as tensor_scalar (4x mode) + tensor_tensor (2x) on
  DVE; scalar_tensor_tensor has no DVE fast mode and Pool cannot run it.
- s: per-chunk squares with sqsum matmuls lagged 3 chunks behind the
  conv loop so the s-accumulation overlaps later chunks' convs.
- per-sample scalar pipeline fused across the job group ([128, 4*J]-wide
  tiny DVE ops) with 1 Newton rsqrt step (error budget: l2 3.5e-3 vs
  2e-2 gate).
- relu as two wide slabs: tensor_scalar_max on DVE + Relu
  activation on Act (splits the z-phase tail across engines).
- software-pipelined emission: groups advance (front, scalars, back)
  staggered LAG steps apart, youngest group emitted first, so the
  in-order engine queues always hold the next group's dependency-free
  conv/copy work ahead of this group's long cross-engine scalar chains.
"""

import os
import sys

for _p in ("/opt/trn_rl_repo", "/root/.axon_site/_ro/trn_rl_repo"):
    if _p not in sys.path and os.path.isdir(_p):
        sys.path.append(_p)

import numpy as np
import ml_dtypes
import concourse.bacc as bacc
import concourse.mybir as mybir
import concourse.tile as tile
from concourse import bass_utils

F32 = mybir.dt.float32
BF16 = mybir.dt.bfloat16
I32 = mybir.dt.int32
OP = mybir.AluOpType
ACT = mybir.ActivationFunctionType
NPBF = ml_dtypes.bfloat16

NCORES = 8
BATCH = 32768
INSIZE = 1024
FLEN = 64
NUM_LAYERS = 4
ROWS_PER_CORE = BATCH // NCORES          # 4096
NB = 512                                  # batch columns per tile
NTILES = ROWS_PER_CORE // NB              # 8
MAXNORM = 1.0 - 4e-3
COEF_Y = 1.0 - MAXNORM * MAXNORM

# engine split knob: z-STT chunk c goes to DVE when c % ZPOOL == 0, else Pool
ZPOOL = int(os.environ.get("HD_ZPOOL", "1"))
# last-layer m-mult: "dve" = all DVE, "alt" = alternate DVE/Pool
LASTM = os.environ.get("HD_LASTM", "dve")
# PSUM->SBUF copies: chunk c on DVE tensor_copy when c % COPYD == COPYD-1
# (COPYD=0 disables: all copies on Act)
COPYD = int(os.environ.get("HD_COPYD", "0"))
# z op form when on DVE: "stt" fused, "tstt" = tensor_scalar + tensor_tensor
ZMODE = os.environ.get("HD_ZMODE", "tstt")
# tb = dbc*ycol engine: "dve" (tensor_scalar) or "act" (activation scale)
TBENG = os.environ.get("HD_TBENG", "dve")
# relu engine: "dve" or "pool"
RELUENG = os.environ.get("HD_RELUENG", "act2")
# jobs interleaved per group (pipeline depth)
PAIR = int(os.environ.get("HD_PAIR", "2"))
# Newton iterations for rsqrt (1 is plenty for the 2e-2 gate)
NEWTON = int(os.environ.get("HD_NEWTON", "1"))
# stats matmuls: 0 = right after each job's front, 1 = after all fronts
SPLITSTATS = int(os.environ.get("HD_SPLITSTATS", "1"))
# squares: 0 = two slab halves per layer, 1 = per-chunk (finer deps so the
# sqsum matmul chain overlaps the conv of later chunks)
SQCHUNK = int(os.environ.get("HD_SQCHUNK", "1"))
# broadcast delta/m rows: 0 = gpsimd partition_broadcast, 1 = PE rank-1
# matmul + DVE copy (keeps Pool off the critical path)
BCPE = int(os.environ.get("HD_BCPE", "0"))
# first relu slab chunk count (gates next layer's first convs); 0 = nout//2
RELU1 = int(os.environ.get("HD_RELU1", "0"))
# split last-layer output DMA into two halves (first half streams early)
OUTSPLIT = int(os.environ.get("HD_OUTSPLIT", "0"))
# emit beta (d) matmul chain before the conv matmuls in the PE stream
BETAEARLY = int(os.environ.get("HD_BETAEARLY", "0"))

LIN = [INSIZE + FLEN * i for i in range(NUM_LAYERS)]         # 1024 1088 1152 1216
LOUT = [l + FLEN for l in LIN]                                # 1088 1152 1216 1280
NIN = [(l + 127) // 128 for l in LIN]                         # 8 9 9 10
NOUT = [(l + 127) // 128 for l in LOUT]                       # 9 9 10 10


def host_prep(w, b_list):
    """Replicated parameter layouts (repacking + tiny weight correlations)."""
    prep = {}
    WF = np.zeros((NUM_LAYERS, 128, 128), np.float32)
    WU = np.zeros((NUM_LAYERS, 128, 128), np.float32)  # rows 64-127 used
    for i in range(NUM_LAYERS):
        for k in range(128):
            for r in range(128):
                t = r - k
                if 0 <= t < FLEN:
                    WF[i, k, r] = w[i, t]
        for k in range(64):
            for r in range(128):
                t = r + 64 - k
                if 0 <= t < FLEN:
                    WU[i, 64 + k, r] = w[i, t]
    prep["wf"] = WF.astype(NPBF)
    prep["wu"] = WU.astype(NPBF)

    nmax_in, nmax_out = max(NIN), max(NOUT)
    beta = np.zeros((NUM_LAYERS, 128, nmax_in), np.float32)
    ycol = np.zeros((NUM_LAYERS, 128, nmax_out), np.float32)
    y2cx = np.zeros((NUM_LAYERS, 128, 2), np.float32)
    for i in range(NUM_LAYERS):
        b64 = b_list[i].astype(np.float64)
        bt = np.correlate(b64, w[i].astype(np.float64), mode="valid")[: LIN[i]]
        bpad = np.zeros(NIN[i] * 128)
        bpad[: LIN[i]] = bt
        beta[i, :, : NIN[i]] = bpad.reshape(NIN[i], 128).T.astype(np.float32)
        ypad = np.zeros(NOUT[i] * 128)
        ypad[: LOUT[i]] = b64
        ycol[i, :, : NOUT[i]] = ypad.reshape(NOUT[i], 128).T.astype(np.float32)
        y2 = np.float32(np.sum(b_list[i].astype(np.float32) ** 2, dtype=np.float32))
        y2cx[i, :, 0] = np.float32(1.0) + np.float32(MAXNORM * MAXNORM) * y2
        y2cx[i, :, 1] = np.float32(1.0) + y2
    prep["beta"] = beta.astype(NPBF)
    prep["ycol"] = ycol
    prep["y2cx"] = y2cx
    prep["ones"] = np.ones((128, 1), NPBF)
    prep["onesr"] = np.ones((1, 128), NPBF)
    prep["id128"] = np.eye(128, dtype=np.float32)
    prep["id128b"] = np.eye(128, dtype=NPBF)
    return prep


def build_program(ntiles=NTILES, reps=1, loop=0):
    nc = bacc.Bacc("TRN2", target_bir_lowering=False, debug=False)
    nmax_in, nmax_out = max(NIN), max(NOUT)
    ncols = ntiles * NB

    hkT = nc.dram_tensor("hkT", [INSIZE, ncols], BF16, kind="ExternalInput")
    d_wf = nc.dram_tensor("wf", [NUM_LAYERS, 128, 128], BF16, kind="ExternalInput")
    d_wu = nc.dram_tensor("wu", [NUM_LAYERS, 128, 128], BF16, kind="ExternalInput")
    d_beta = nc.dram_tensor("beta", [NUM_LAYERS, 128, nmax_in], BF16, kind="ExternalInput")
    d_ycol = nc.dram_tensor("ycol", [NUM_LAYERS, 128, nmax_out], F32, kind="ExternalInput")
    d_y2cx = nc.dram_tensor("y2cx", [NUM_LAYERS, 128, 2], F32, kind="ExternalInput")
    d_ones = nc.dram_tensor("ones", [128, 1], BF16, kind="ExternalInput")
    d_onesr = nc.dram_tensor("onesr", [1, 128], BF16, kind="ExternalInput")
    d_id = nc.dram_tensor("id128", [128, 128], F32, kind="ExternalInput")
    d_idb = nc.dram_tensor("id128b", [128, 128], BF16, kind="ExternalInput")
    outT = nc.dram_tensor("outT", [LOUT[-1], ncols], BF16, kind="ExternalOutput")

    hk_v = hkT.rearrange("(c p) n -> p c n", p=128)
    out_v = outT.rearrange("(c p) n -> p c n", p=128)

    with tile.TileContext(nc) as tc:
        with (
            tc.tile_pool(name="singles", bufs=1) as singles,
            tc.tile_pool(name="u0p", bufs=2 * PAIR) as u0p,
            tc.tile_pool(name="acts", bufs=int(os.environ.get("HD_ACTS", str(2 * PAIR + 2)))) as acts,
            tc.tile_pool(name="cvsb", bufs=2 * PAIR + 1) as cvsbp,
            tc.tile_pool(name="sqp", bufs=int(os.environ.get("HD_SQBUFS", "2"))) as sqp,
            tc.tile_pool(name="outp", bufs=PAIR) as outp,
            tc.tile_pool(name="smallp", bufs=2) as smallp,
            tc.tile_pool(name="bcsb", bufs=PAIR + 1) as bcsbp,
            tc.tile_pool(name="tbp", bufs=3) as tbp,
            tc.tile_pool(name="cvps", bufs=int(os.environ.get("HD_CVPS", "2")), space="PSUM") as cvps,
            tc.tile_pool(name="stps", bufs=4, space="PSUM") as stps,
            tc.tile_pool(name="tinyp", bufs=int(os.environ.get("HD_TINYPS", "2")), space="PSUM") as tinyps,
        ):
            s_wf = singles.tile([128, NUM_LAYERS, 128], BF16, tag="wf")
            s_wu = singles.tile([128, NUM_LAYERS, 128], BF16, tag="wu")
            s_beta = singles.tile([128, NUM_LAYERS, nmax_in], BF16, tag="beta")
            s_ycol = singles.tile([128, NUM_LAYERS, nmax_out], F32, tag="ycol")
            s_y2cx = singles.tile([128, NUM_LAYERS, 2], F32, tag="y2cx")
            s_ones = singles.tile([128, 1], BF16, tag="ones")
            s_onesr = singles.tile([1, 128], BF16, tag="onesr")
            s_id = singles.tile([128, 128], F32, tag="id")
            s_idb = singles.tile([128, 128], BF16, tag="idb")
            nc.sync.dma_start(out=s_wf, in_=d_wf.rearrange("l p m -> p l m"))
            nc.sync.dma_start(out=s_wu, in_=d_wu.rearrange("l p m -> p l m"))
            nc.sync.dma_start(out=s_beta, in_=d_beta.rearrange("l p m -> p l m"))
            nc.sync.dma_start(out=s_ycol, in_=d_ycol.rearrange("l p m -> p l m"))
            nc.sync.dma_start(out=s_y2cx, in_=d_y2cx.rearrange("l p m -> p l m"))
            nc.sync.dma_start(out=s_ones, in_=d_ones[:])
            nc.sync.dma_start(out=s_onesr, in_=d_onesr[:])
            nc.sync.dma_start(out=s_id, in_=d_id[:])
            nc.sync.dma_start(out=s_idb, in_=d_idb[:])

            def emit_front(job, li):
                u = job["u"]
                lin, lout = LIN[li], LOUT[li]
                nin, nout = NIN[li], NOUT[li]
                partial = lout % 128 != 0
                wf_l = s_wf[:, li, :]
                wu_l = s_wu[:, li, :]

                st = stps.tile([33, NB], F32, tag="st")
                stats_s, stats_d = st[0:1, :], st[32:33, :]

                if BETAEARLY:
                    # d = <u, beta> is ready at layer entry; emit it ahead
                    # of the convs so it never queues behind their tail
                    for ch in range(nin):
                        k = 128 if (ch + 1) * 128 <= lin else 64
                        nc.tensor.matmul(
                            stats_d,
                            s_beta[0:k, li, ch: ch + 1],
                            u[0:k, ch, :],
                            start=(ch == 0), stop=(ch == nin - 1),
                            tile_position=(0, 32),
                        )

                # ---- conv chunks -> PSUM -> Act copy to bf16 slab ----
                cvsb = cvsbp.tile([128, nout, NB], BF16, tag="cvsb")
                sq = sqp.tile([128, nout, NB], BF16, tag="sq")
                if partial:
                    nc.vector.memset(cvsb[64:128, nout - 1, :], 0.0)

                def emit_sqsum(c):
                    nc.tensor.matmul(
                        stats_s, s_ones, sq[:, c, :],
                        start=(c == 0), stop=(c == nout - 1),
                        tile_position=(0, 0),
                    )

                for c in range(nout):
                    outv = 128 if (c + 1) * 128 <= lout else 64
                    pcv = cvps.tile([128, NB], F32, tag="cv")
                    mms = []
                    if c > 0 and 128 * c <= lin:
                        # upper 64 rows of input chunk c-1 (WU rows 0-63
                        # are zero so the full-K matmul only reads the
                        # upper half)
                        mms.append((wu_l[:, 0:outv], u[:, c - 1, :]))
                    if 128 * (c + 1) <= lin:
                        mms.append((wf_l[:, 0:outv], u[:, c, :]))
                    elif 128 * c + 64 <= lin:
                        mms.append((wf_l[0:64, 0:outv], u[0:64, c, :]))
                    assert mms
                    for mi, (lhs, rhs) in enumerate(mms):
                        nc.tensor.matmul(
                            pcv[:outv, :], lhs, rhs,
                            start=(mi == 0), stop=(mi == len(mms) - 1),
                            tile_position=(0, 0),
                        )
                    if COPYD and c % COPYD == COPYD - 1:
                        nc.vector.tensor_copy(cvsb[:outv, c, :], pcv[:outv, :])
                    else:
                        nc.scalar.copy(cvsb[:outv, c, :], pcv[:outv, :])
                    if SQCHUNK:
                        # per-chunk square + lagged sqsum matmul: the s
                        # accumulation overlaps later chunks' convs
                        nc.vector.tensor_tensor(
                            sq[:, c, :], cvsb[:, c, :], cvsb[:, c, :], OP.mult)
                        if c >= 3:
                            emit_sqsum(c - 3)
                        if c == nout - 1:
                            for cc in range(max(nout - 3, 0), nout):
                                emit_sqsum(cc)

                if not SQCHUNK:
                    half = nout // 2
                    nc.vector.tensor_tensor(
                        sq[:, 0:half, :], cvsb[:, 0:half, :], cvsb[:, 0:half, :],
                        OP.mult)
                    nc.vector.tensor_tensor(
                        sq[:, half:nout, :], cvsb[:, half:nout, :],
                        cvsb[:, half:nout, :], OP.mult)

                job["cvsb"] = cvsb
                job["st"] = st
                job["sq"] = sq

            def emit_stats(job, li):
                """s/d ones-matmul chains — emitted after ALL jobs' convs so
                they don't block the next job's conv in the PE queue."""
                u = job["u"]
                lin, lout = LIN[li], LOUT[li]
                nin, nout = NIN[li], NOUT[li]
                st, sq = job["st"], job["sq"]
                stats_s, stats_d = st[0:1, :], st[32:33, :]
                if not SQCHUNK:
                    for c in range(nout):
                        nc.tensor.matmul(
                            stats_s, s_ones, sq[:, c, :],
                            start=(c == 0), stop=(c == nout - 1),
                            tile_position=(0, 0),
                        )
                for ch in ([] if BETAEARLY else range(nin)):
                    k = 128 if (ch + 1) * 128 <= lin else 64
                    nc.tensor.matmul(
                        stats_d,
                        s_beta[0:k, li, ch: ch + 1],
                        u[0:k, ch, :],
                        start=(ch == 0), stop=(ch == nin - 1),
                        tile_position=(0, 32),
                    )

            def emit_scalars(jobs, li):
                """Fused per-sample scalar pipeline for the whole job group:
                one set of [128, 4*J]-wide DVE ops instead of J serial
                pipelines. Produces dbc (and mbc on the last layer) per job.
                """
                last = li == NUM_LAYERS - 1
                J = len(jobs)
                with tc.high_priority():
                    sd_sb = smallp.tile([1, J * 2 * NB], BF16, tag="sdsb")
                    for j, job in enumerate(jobs):
                        st = job["st"]
                        nc.scalar.copy(
                            sd_sb[0:1, (2 * j) * NB: (2 * j + 1) * NB], st[0:1, :])
                        nc.scalar.copy(
                            sd_sb[0:1, (2 * j + 1) * NB: (2 * j + 2) * NB],
                            st[32:33, :])
                    scr = tinyps.tile([128, 16 * J + 512], BF16, tag="tiny")
                    for p in range(8 * J):
                        nc.tensor.transpose(
                            scr[:, 2 * p: 2 * p + 1],
                            sd_sb[0:1, 128 * p: 128 * p + 128],
                            s_idb[:1, :1],
                        )
                    W = 4 * J
                    sc = smallp.tile([128, 10 * W], F32, tag="sc")
                    sci = sc.bitcast(I32)
                    # scr col order per job: s(4), d(4); gather S then D
                    scr_v = scr[:, 0: 16 * J].rearrange(
                        "p (j t c) -> p j t c", j=J, t=2, c=8)
                    nc.vector.tensor_copy(
                        sc[:, 0:W].rearrange("p (j c) -> p j c", j=J),
                        scr_v[:, :, 0, 0:8:2])
                    nc.vector.tensor_copy(
                        sc[:, W: 2 * W].rearrange("p (j c) -> p j c", j=J),
                        scr_v[:, :, 1, 0:8:2])
                    S, D = sc[:, 0:W], sc[:, W: 2 * W]
                    Si = sci[:, 0:W]

                    def colw(k):
                        return sc[:, (2 + k) * W: (3 + k) * W]

                    r, t1, t2, sqs, t0, den, cx, P = (colw(k) for k in range(8))
                    ri32 = sci[:, 2 * W: 3 * W]
                    # rsqrt(s): quake seed + 1 Newton step (err ~0.2%, well
                    # inside the 2e-2 gate; halves the serial chain)
                    nc.vector.tensor_scalar(
                        ri32, Si, 1, None, OP.logical_shift_right)
                    nc.vector.tensor_scalar(
                        ri32, ri32, 0x5F3759DF, -1, OP.subtract, OP.mult)
                    for _ in range(NEWTON):
                        nc.vector.tensor_tensor(t1, S, r, OP.mult)
                        nc.vector.tensor_tensor(t2, t1, r, OP.mult)
                        nc.vector.tensor_scalar(t2, t2, -0.5, 1.5, OP.mult, OP.add)
                        nc.vector.tensor_tensor(r, r, t2, OP.mult)
                    nc.vector.tensor_tensor(sqs, S, r, OP.mult)         # sqrt(s)
                    nc.vector.tensor_tensor(t0, D, r, OP.mult)          # d / sqrt(s)
                    nc.vector.tensor_scalar(
                        den, t0, 2.0 * MAXNORM, s_y2cx[:, li, 0:1], OP.mult, OP.add)
                    nc.vector.tensor_scalar(
                        cx, t0, 2.0 * MAXNORM, s_y2cx[:, li, 1:2], OP.mult, OP.add)
                    dm = smallp.tile([128, 2 * W], BF16, tag="dm")
                    nc.vector.reciprocal(P, cx)
                    nc.vector.scalar_tensor_tensor(
                        dm[:, 0:W], sqs, COEF_Y / MAXNORM, P,
                        OP.mult, OP.mult)  # (C/M) * sqrt(s)/cx
                    if last:
                        nc.vector.reciprocal(t1, den)
                        nc.vector.tensor_tensor(t2, cx, r, OP.mult)
                        nc.vector.scalar_tensor_tensor(
                            dm[:, W: 2 * W], t2, MAXNORM, t1,
                            OP.mult, OP.mult)  # M*cx*r/den

                    def bc_build(col0):
                        btp = scr[0:1, 16 * J: 16 * J + 512]
                        for k in range(4):
                            nc.tensor.transpose(
                                btp[0:1, 128 * k: 128 * (k + 1)],
                                dm[:, col0 + k: col0 + k + 1], s_idb)
                        rws = smallp.tile([1, 512], BF16, tag="rows")
                        nc.vector.tensor_copy(rws[0:1, :], btp[0:1, :])
                        bcs = bcsbp.tile([128, NB], BF16, tag="bcs")
                        if BCPE:
                            bcps = cvps.tile([128, NB], F32, tag="cv")
                            nc.tensor.matmul(
                                bcps, s_onesr, rws[0:1, :],
                                start=True, stop=True, tile_position=(0, 0))
                            nc.vector.tensor_copy(bcs[:, :], bcps)
                        else:
                            nc.gpsimd.partition_broadcast(bcs[:, :], rws[0:1, :])
                        return bcs

                    for j, job in enumerate(jobs):
                        job["dbc"] = bc_build(4 * j)
                        job["mbc"] = bc_build(W + 4 * j) if last else None

            def emit_back(job, li):
                """z = cvsb + dbc*y, relu, (last: *mbc + DMA out)."""
                lin, lout = LIN[li], LOUT[li]
                nout = NOUT[li]
                last = li == NUM_LAYERS - 1
                cvsb, dbc, mbc = job["cvsb"], job["dbc"], job["mbc"]

                dstp = outp if last else acts
                un = dstp.tile([128, nout, NB], BF16,
                               tag="out" if last else "un")
                for c in range(nout):
                    if ZPOOL < 0:
                        on_dve = (c % (-ZPOOL)) != 0
                    else:
                        on_dve = (c % ZPOOL) == 0
                    if on_dve:
                        if ZMODE == "stt":
                            # fused on DVE (STT is DVE-only in the hw ISA)
                            nc.vector.scalar_tensor_tensor(
                                un[:, c, :], dbc, s_ycol[:, li, c: c + 1],
                                cvsb[:, c, :], OP.mult, OP.add)
                        else:
                            tb = tbp.tile([128, NB], BF16, tag="tb")
                            if TBENG == "act":
                                nc.scalar.mul(tb, dbc, s_ycol[:, li, c: c + 1])
                            else:
                                nc.vector.tensor_scalar(
                                    tb, dbc, s_ycol[:, li, c: c + 1], None, OP.mult)
                            nc.vector.tensor_tensor(
                                un[:, c, :], tb, cvsb[:, c, :], OP.add)
                    else:
                        # split: tb on DVE (4x tensor_scalar), add on Pool
                        tb = tbp.tile([128, NB], BF16, tag="tb")
                        nc.vector.tensor_scalar(
                            tb, dbc, s_ycol[:, li, c: c + 1], None, OP.mult)
                        nc.gpsimd.tensor_tensor(
                            un[:, c, :], tb, cvsb[:, c, :], OP.add)

                # relu: wide slab halves (4x tensor_scalar mode on DVE)
                reng = nc.gpsimd if RELUENG == "pool" else nc.vector
                half_r = RELU1 if RELU1 else nout // 2
                reng.tensor_scalar_max(
                    un[:, 0:half_r, :], un[:, 0:half_r, :], 0.0)
                if RELUENG == "act2":
                    nc.scalar.activation(
                        un[:, half_r:nout, :], un[:, half_r:nout, :], ACT.Relu)
                else:
                    reng.tensor_scalar_max(
                        un[:, half_r:nout, :], un[:, half_r:nout, :], 0.0)

                if not last:
                    job["u"] = un
                else:
                    if OUTSPLIT:
                        h = nout // 2
                        for c in range(h):
                            nc.vector.tensor_tensor(
                                un[:, c, :], un[:, c, :], mbc, OP.mult)
                        nc.sync.dma_start(
                            out=out_v[:, 0:h, job["ncol"]], in_=un[:, 0:h, :])
                        for c in range(h, nout):
                            nc.vector.tensor_tensor(
                                un[:, c, :], un[:, c, :], mbc, OP.mult)
                        nc.sync.dma_start(
                            out=out_v[:, h:nout, job["ncol"]], in_=un[:, h:nout, :])
                    else:
                        for c in range(nout):
                            eng = nc.vector if (LASTM == "dve" or c % 2 == 0) else nc.gpsimd
                            eng.tensor_tensor(un[:, c, :], un[:, c, :], mbc, OP.mult)
                        nc.sync.dma_start(out=out_v[:, :, job["ncol"]], in_=un)

            def emit_jobs():
                njobs = ntiles * reps
                # Software-pipelined emission: each group's (front, scalars,
                # back) stages advance one layer behind the next group's
                # front, so the in-order PE queue always has the following
                # group's conv matmuls ahead of this group's pipeline
                # transposes (which depend on long cross-engine chains).
                groups = []
                for j0 in range(0, njobs, PAIR):
                    jobs = []
                    for j in range(j0, min(j0 + PAIR, njobs)):
                        jj = j % ntiles
                        ncol = slice(jj * NB, (jj + 1) * NB)
                        jobs.append({"ncol": ncol})
                    groups.append(jobs)

                def load(jobs):
                    for job in jobs:
                        u = u0p.tile([128, NIN[0], NB], BF16, tag="u0")
                        nc.sync.dma_start(out=u, in_=hk_v[:, :, job["ncol"]])
                        job["u"] = u

                def stage(jobs, li, phase):
                    if phase == 0:
                        if SPLITSTATS:
                            for job in jobs:
                                emit_front(job, li)
                            for job in jobs:
                                emit_stats(job, li)
                        else:
                            for job in jobs:
                                emit_front(job, li)
                                emit_stats(job, li)
                    elif phase == 1:
                        emit_scalars(jobs, li)
                    else:
                        for job in jobs:
                            emit_back(job, li)

                # schedule: group g runs (li, phase) at step 3*li + phase;
                # group g+1 lags LAG steps behind group g. LAG=4 keeps
                # phases offset so one group's fronts interleave another's
                # scalars; youngest group emits first each step so its
                # (dependency-free) fronts precede older groups' transposes
                # in the in-order engine queues.
                LAG = int(os.environ.get("HD_LAG", "2"))
                nsteps = 3 * NUM_LAYERS
                total = nsteps + LAG * (len(groups) - 1)
                loaded = set()
                for t in range(-1, total + LAG):
                    for g in range(len(groups) - 1, -1, -1):
                        jobs = groups[g]
                        step = t - LAG * g
                        if step == -1 and g not in loaded:
                            load(jobs)
                            loaded.add(g)
                        elif 0 <= step < nsteps:
                            li, phase = divmod(step, 3)
                            stage(jobs, li, phase)

            if loop:
                with tc.For_i(0, loop):
                    emit_jobs()
            else:
                emit_jobs()

    nc.compile()
    return nc


_NC_CACHE = {}


def _get_program(ntiles=NTILES):
    if ntiles not in _NC_CACHE:
        _NC_CACHE[ntiles] = build_program(ntiles)
    return _NC_CACHE[ntiles]


def prep_hkT(hk_rows):
    """Host-side layout prep for one core's batch rows -> hkT DRAM tensor."""
    return np.ascontiguousarray(hk_rows.T).astype(NPBF)


def kernel(**inputs):
    hk = np.asarray(inputs["hk"], dtype=np.float32)
    w = np.asarray(inputs["w"], dtype=np.float32)
    b_list = [np.asarray(inputs[f"b{i}"], dtype=np.float32) for i in range(NUM_LAYERS)]

    prep = host_prep(w, b_list)
    nc = _get_program()

    in_maps = []
    for c in range(NCORES):
        rows = slice(c * ROWS_PER_CORE, (c + 1) * ROWS_PER_CORE)
        m = dict(prep)
        m["hkT"] = prep_hkT(hk[rows])
        in_maps.append(m)

    res = bass_utils.run_bass_kernel_spmd(nc, in_maps, list(range(NCORES)))
    outs = [
        np.asarray(res.results[c]["outT"]).astype(np.float32).T
        for c in range(NCORES)
    ]
    return np.ascontiguousarray(np.concatenate(outs, axis=0))
